# revision 11
# baseline (speedup 1.0000x reference)
"""Trainium2 Bass kernel for nn_GAT_WLN (GNN message passing, 8 NeuronCores).

Strategy (graph/data parallel, hinted):
  - Nodes sharded 512/core; edges sharded by destination node.
  - The big [E, H+D] @ [H+D, H] edge matmul is factored algebraically:
    concat([h[src], ea]) @ W.T == (h @ Wa.T)[src] + (ea @ Wb.T), so edge work
    becomes node-level matmuls + indirect-DMA row gathers + one-hot
    scatter-matmuls (edges pre-sorted by dst into 128-node windows on host).
  - Self-loops for GAT are appended to the edge stream on host; softmax is
    computed without the max-subtraction (validated: |e| < ~2, fp32 exact).
  - Three AllGathers: P [4096,256], [R|g|a_s] [4096,520], q [4096,5].
  - Pairwise map q[x]+q[y]: per core a [512,4096,5] slab (42 MB) built by
    rank-6 matmuls against a host-precomputed interleave pattern, drained
    PSUM->SBUF on ACT/DVE, DMA'd out; diagonal -1 rows via indirect scatter.
"""
import numpy as np
import ml_dtypes

N, E = 4096, 32768
F, D, H, C = 82, 6, 256, 5
SLOPE = 0.2
NCORES = 8
NPC = N // NCORES          # 512 nodes per core
WIN = 128                  # dst window
WPC = NPC // WIN           # 4 windows per core

BF16 = ml_dtypes.bfloat16

_cache = {}


# ----------------------------------------------------------------------------
# host-side preprocessing
# ----------------------------------------------------------------------------
def _prep(edge_index, edge_attr):
    src = np.asarray(edge_index[0], dtype=np.int64)
    dst = np.asarray(edge_index[1], dtype=np.int64)
    ea = np.asarray(edge_attr, dtype=np.float32)

    # group edge ids by (core, window); self-loops appended per window
    order = np.argsort(dst, kind="stable")
    srcs, dsts = src[order], dst[order]
    eas = ea[order]

    counts = np.zeros((NCORES, WPC), dtype=np.int64)
    groups = [[None] * WPC for _ in range(NCORES)]
    gidx = dsts // WIN                     # global window id 0..31
    bounds = np.searchsorted(gidx, np.arange(NCORES * WPC + 1))
    for r in range(NCORES):
        for w in range(WPC):
            gw = r * WPC + w
            lo, hi = bounds[gw], bounds[gw + 1]
            groups[r][w] = (lo, hi)
            counts[r, w] = (hi - lo) + WIN   # + self loops

    T_w = int(-(-counts.max() // 128))     # tiles per window
    EPW = T_w * 128
    EP = WPC * EPW
    T_tot = WPC * T_w

    cores = []
    for r in range(NCORES):
        src_sb = np.zeros((128, T_tot), np.int32)
        eaT7 = np.zeros((7, EP), np.float32)
        ohBC = np.zeros((128, T_tot * 128), np.float32)
        ohGAT = np.zeros((128, T_tot * 128), np.float32)
        ohGATT = np.zeros((128, T_tot * 128), np.float32)
        for w in range(WPC):
            lo, hi = groups[r][w]
            n_real = hi - lo
            base = w * EPW
            e_pos = base + np.arange(n_real)
            s_pos = base + n_real + np.arange(WIN)     # self loops
            # real edges
            src_sb[e_pos % 128, e_pos // 128] = srcs[lo:hi]
            eaT7[:D, e_pos] = eas[lo:hi].T
            eaT7[6, e_pos] = 1.0
            nloc = (dsts[lo:hi] % WIN).astype(np.int64)
            ohBC[e_pos % 128, (e_pos // 128) * 128 + nloc] = 1.0
            ohGAT[e_pos % 128, (e_pos // 128) * 128 + nloc] = 1.0
            ohGATT[nloc, (e_pos // 128) * 128 + (e_pos % 128)] = 1.0
            # self loops: node ids r*NPC + w*WIN + [0..127]
            self_ids = r * NPC + w * WIN + np.arange(WIN)
            src_sb[s_pos % 128, s_pos // 128] = self_ids
            nl = np.arange(WIN)
            ohGAT[s_pos % 128, (s_pos // 128) * 128 + nl] = 1.0
            ohGATT[nl, (s_pos // 128) * 128 + (s_pos % 128)] = 1.0
        iloc = np.arange(NPC)
        diag_sb = ((iloc * N) + (r * NPC + iloc)).astype(np.int32).reshape(WPC, 128).T
        cores.append(dict(
            src_sb=src_sb,
            eaT7=eaT7.astype(BF16),
            ohBC=ohBC.astype(BF16),
            ohGAT=ohGAT.astype(BF16),
            ohGATT=ohGATT.astype(BF16),
            diag_sb=np.ascontiguousarray(diag_sb),
        ))
    return cores, T_w


def _prep_weights(g):
    f32 = np.float32

    def c(a):
        return np.ascontiguousarray(a, dtype=f32)

    def kchunks(wT, nk, m=None):
        # [K, M] -> [128, nk, M]
        K, M = wT.shape
        assert K == nk * 128
        return c(wT.reshape(nk, 128, M).transpose(1, 0, 2))

    W1a = g["wl1_W1"][:, :H]
    W1b = g["wl1_W1"][:, H:]
    out = {}
    out["wlinT"] = c(g["W_lin"].T)                                   # [82, 256]
    out["w1aT"] = kchunks(c(W1a.T), 2)                               # [128,2,256]
    out["w1bT7"] = c(np.vstack([W1b.T, g["wl1_b1"][None, :]])).astype(BF16)
    out["w2T"] = kchunks(c(g["wl1_W2"].T), 4)                        # [128,4,256]
    out["b2c"] = c(g["wl1_b2"].reshape(2, 128).T)                    # [128, 2]
    out["w3T"] = kchunks(c(g["wl2_W3"].T), 2)
    out["b3c"] = c(g["wl2_b3"].reshape(2, 128).T)
    out["w2c7"] = c(np.vstack([g["wl2_W2"].T,
                               g["wl2_b2"][None, :]])).astype(BF16)
    out["gatwT"] = kchunks(c(g["gat_W"].T), 2)
    out["asrcc"] = c(g["gat_asrc"].reshape(2, 128).T)                # [128, 2]
    out["adstc"] = c(g["gat_adst"].reshape(2, 128).T)
    out["wl2T"] = kchunks(c(g["W_lin2"].T), 2)
    out["wl3T"] = kchunks(c(g["W_lin3"].T), 2)                       # [128,2,5]
    out["qconstc"] = c(((g["gat_b"] @ g["W_lin2"].T) @ g["W_lin3"].T)[:, None])
    # pairwise interleave pattern rows 0..4: pat5[c, j*5+c'] = (c == c')
    out["pat5"] = np.ascontiguousarray(
        np.tile(np.eye(5, dtype=f32), N).astype(BF16))               # [5, 5N] bf16
    return out


# ----------------------------------------------------------------------------
# device program
# ----------------------------------------------------------------------------
def _build(T_w):
    import concourse.bass as bass
    import concourse.tile as tile
    from concourse import bacc, mybir
    from concourse.bass import IndirectOffsetOnAxis, ts
    from concourse.masks import make_identity

    f32 = mybir.dt.float32
    bf16 = mybir.dt.bfloat16
    i32 = mybir.dt.int32
    AF = mybir.ActivationFunctionType
    OP = mybir.AluOpType

    T_tot = WPC * T_w
    EP = T_tot * 128
    JCH = 512 * C          # 2560 output cols per chunk
    NJC = N // 512         # 8 chunks per row-tile

    nc = bacc.Bacc("TRN2", target_bir_lowering=False, debug=False,
                   enable_asserts=False, num_devices=NCORES)

    din = {}

    def inp(name, shape, dt=f32):
        din[name] = nc.dram_tensor(name, list(shape), dt, kind="ExternalInput").ap()
        return din[name]

    d_xT = inp("xT", [F, NPC])
    d_wlinT = inp("wlinT", [F, H])
    d_w1aT = inp("w1aT", [128, 2, H])
    d_w1bT7 = inp("w1bT7", [7, H], bf16)
    d_w2T = inp("w2T", [128, 4, H])
    d_b2c = inp("b2c", [128, 2])
    d_w3T = inp("w3T", [128, 2, H])
    d_b3c = inp("b3c", [128, 2])
    d_w2c7 = inp("w2c7", [7, H], bf16)
    d_gatwT = inp("gatwT", [128, 2, H])
    d_asrcc = inp("asrcc", [128, 2])
    d_adstc = inp("adstc", [128, 2])
    d_wl2T = inp("wl2T", [128, 2, H])
    d_wl3T = inp("wl3T", [128, 2, C])
    d_qconstc = inp("qconstc", [C, 1])
    d_pat5 = inp("pat5", [5, C * N], bf16)
    d_src = inp("src_sb", [128, T_tot], i32)
    d_ea7 = inp("eaT7", [7, EP], bf16)
    d_ohBC = inp("ohBC", [128, T_tot * 128], bf16)
    d_ohG = inp("ohGAT", [128, T_tot * 128], bf16)
    d_ohGT = inp("ohGATT", [128, T_tot * 128], bf16)
    d_diag = inp("diag_sb", [128, WPC], i32)

    out_h = nc.dram_tensor("out", [NPC * N, C], f32, kind="ExternalOutput")
    out_flat = out_h.ap()                                  # [512*4096, 5]
    out3 = out_flat.rearrange("(i j) c -> i j c", i=NPC)   # [512, 4096, 5]

    from contextlib import ExitStack
    with tile.TileContext(nc) as tc, ExitStack() as ctx:
        const = ctx.enter_context(tc.tile_pool(name="const", bufs=1))
        nodes = ctx.enter_context(tc.tile_pool(name="nodes", bufs=1))
        epool = ctx.enter_context(tc.tile_pool(name="edge", bufs=3))
        pwpool = ctx.enter_context(tc.tile_pool(name="pw", bufs=1))
        psum = ctx.enter_context(tc.tile_pool(name="psum", bufs=1, space="PSUM"))
        dram = ctx.enter_context(tc.tile_pool(name="dram", bufs=1, space="DRAM"))

        _pmm_n = [0]

        def pmm(shape=(128, 512)):
            _pmm_n[0] += 1
            return psum.tile(list(shape), f32, tag="mm", bufs=4,
                             name=f"pmm{_pmm_n[0]}")

        # ---- resident constants ----
        def cload(name, ap, dt=f32):
            t = const.tile(list(ap.shape), dt, name=name)
            nc.sync.dma_start(out=t[:], in_=ap)
            return t

        sb_xT = cload("sb_xT", d_xT)
        sb_wlinT = cload("sb_wlinT", d_wlinT)
        sb_w1aT = cload("sb_w1aT", d_w1aT)
        sb_w1b7 = cload("sb_w1b7", d_w1bT7, bf16)
        sb_w2T = cload("sb_w2T", d_w2T)
        sb_b2 = cload("sb_b2", d_b2c)
        sb_w3T = cload("sb_w3T", d_w3T)
        sb_b3 = cload("sb_b3", d_b3c)
        sb_w2c7 = cload("sb_w2c7", d_w2c7, bf16)
        sb_gatwT = cload("sb_gatwT", d_gatwT)
        sb_asrc = cload("sb_asrc", d_asrcc)
        sb_adst = cload("sb_adst", d_adstc)
        sb_wl2T = cload("sb_wl2T", d_wl2T)
        sb_wl3T = cload("sb_wl3T", d_wl3T)
        sb_qconst = cload("sb_qconst", d_qconstc)
        sb_src = cload("sb_src", d_src, i32)
        sb_ea7 = cload("sb_ea7", d_ea7, bf16)
        sb_ohBC = cload("sb_ohBC", d_ohBC, bf16)
        sb_ohG = cload("sb_ohG", d_ohG, bf16)
        sb_ohGT = cload("sb_ohGT", d_ohGT, bf16)
        sb_diag = cload("sb_diag", d_diag, i32)

        identity = const.tile([128, 128], f32)
        make_identity(nc, identity[:])
        neg1 = const.tile([128, C], f32)
        nc.vector.memset(neg1[:], -1.0)

        # ---- DRAM collective buffers ----
        ag1_in = dram.tile([NPC, H], f32)
        ag1_out = dram.tile([N, H], f32, addr_space="Shared")
        ag2_in = dram.tile([NPC, 520], f32)
        ag2_out = dram.tile([N, 520], f32, addr_space="Shared")
        ag3_in = dram.tile([NPC, C], bf16)
        ag3_out = dram.tile([N, C], bf16, addr_space="Shared")
        RG = [list(range(NCORES))]

        def transpose_128(dst_ap, src_ap):
            _pmm_n[0] += 1
            p = psum.tile([src_ap.shape[1], src_ap.shape[0]], f32, tag="mm",
                          bufs=4, name=f"ptr{_pmm_n[0]}")
            nc.tensor.transpose(p[:], src_ap,
                                identity[:src_ap.shape[0], :src_ap.shape[0]])
            nc.vector.tensor_copy(dst_ap, p[:])

        # ================= phase A: h0, P(local), AG1 =================
        h0T = nodes.tile([128, 2, NPC], f32)
        for m in range(2):
            p = pmm()
            nc.tensor.matmul(p[:, :NPC], lhsT=sb_wlinT[:, ts(m, 128)],
                             rhs=sb_xT[:], start=True, stop=True)
            nc.scalar.activation(h0T[:, m, :], p[:, :NPC], AF.Relu)

        PT = nodes.tile([128, 2, NPC], f32, tag="ftA")
        for m in range(2):
            p = pmm()
            for kc in range(2):
                nc.tensor.matmul(p[:, :NPC], lhsT=sb_w1aT[:, kc, ts(m, 128)],
                                 rhs=h0T[:, kc, :], start=(kc == 0), stop=(kc == 1))
            nc.vector.tensor_copy(PT[:, m, :], p[:, :NPC])

        P_nm = nodes.tile([128, WPC, H], f32, tag="nmA")
        for w in range(WPC):
            for m in range(2):
                transpose_128(P_nm[:, w, ts(m, 128)], PT[:, m, ts(w, 128)])
            nc.sync.dma_start(out=ag1_in[ts(w, 128), :], in_=P_nm[:, w, :])

        nc.gpsimd.collective_compute("AllGather", OP.bypass, replica_groups=RG,
                                     ins=[ag1_in.opt()], outs=[ag1_out.opt()])

        # ================= phase B edges: msg -> agg -> h1 =================
        agg_nm = nodes.tile([128, WPC, H], f32)
        aggp = [None] * WPC
        for t in range(T_tot):
            w = t // T_w
            if t % T_w == 0:
                aggp[w] = psum.tile([128, H], f32, tag="agg", bufs=2,
                                    name=f"aggB{w}")
            gathP = epool.tile([128, H], f32, tag="gath")
            nc.gpsimd.indirect_dma_start(
                out=gathP[:], out_offset=None, in_=ag1_out[:, :],
                in_offset=IndirectOffsetOnAxis(ap=sb_src[:, t:t + 1], axis=0))
            qp = psum.tile([128, H], f32, tag="mm", bufs=4, name=f"qps{t}")
            nc.tensor.matmul(qp[:], lhsT=sb_ea7[:, ts(t, 128)], rhs=sb_w1b7[:],
                             start=True, stop=True)
            tmp = epool.tile([128, H], f32, tag="tmpB")
            nc.vector.tensor_add(tmp[:], gathP[:], qp[:])
            msg = epool.tile([128, H], bf16, tag="msg")
            nc.scalar.activation(msg[:], tmp[:], AF.Relu)
            nc.tensor.matmul(aggp[w][:], lhsT=sb_ohBC[:, ts(t, 128)], rhs=msg[:],
                             start=(t % T_w == 0), stop=(t % T_w == T_w - 1),
                             skip_group_check=True)
            if t % T_w == T_w - 1:
                nc.scalar.copy(agg_nm[:, w, :], aggp[w][:])

        aggT = nodes.tile([128, 2, NPC], f32, tag="aggT")
        for w in range(WPC):
            for m in range(2):
                transpose_128(aggT[:, m, ts(w, 128)], agg_nm[:, w, ts(m, 128)])

        h1T = nodes.tile([128, 2, NPC], f32)
        for m in range(2):
            p = pmm()
            for kc in range(4):
                rhs = aggT[:, kc, :] if kc < 2 else h0T[:, kc - 2, :]
                nc.tensor.matmul(p[:, :NPC], lhsT=sb_w2T[:, kc, ts(m, 128)],
                                 rhs=rhs, start=(kc == 0), stop=(kc == 3))
            nc.scalar.activation(h1T[:, m, :], p[:, :NPC], AF.Relu,
                                 bias=sb_b2[:, m:m + 1])

        h1_nm = nodes.tile([128, WPC, H], f32)
        for w in range(WPC):
            for m in range(2):
                transpose_128(h1_nm[:, w, ts(m, 128)], h1T[:, m, ts(w, 128)])

        # ================= R, g, a_s, a_d; AG2 =================
        RT = nodes.tile([128, 2, NPC], f32, tag="ftA")
        gT = nodes.tile([128, 2, NPC], f32, tag="ftB")
        for m in range(2):
            p = pmm()
            for kc in range(2):
                nc.tensor.matmul(p[:, :NPC], lhsT=sb_w3T[:, kc, ts(m, 128)],
                                 rhs=h1T[:, kc, :], start=(kc == 0), stop=(kc == 1))
            nc.scalar.activation(RT[:, m, :], p[:, :NPC], AF.Identity,
                                 bias=sb_b3[:, m:m + 1])
            p2 = pmm()
            for kc in range(2):
                nc.tensor.matmul(p2[:, :NPC], lhsT=sb_gatwT[:, kc, ts(m, 128)],
                                 rhs=h1T[:, kc, :], start=(kc == 0), stop=(kc == 1))
            nc.vector.tensor_copy(gT[:, m, :], p2[:, :NPC])

        R_nm = nodes.tile([128, WPC, H], f32, tag="nmA")
        g_nm = nodes.tile([128, WPC, H], f32, tag="nmB")
        as_nm = nodes.tile([128, WPC], f32)
        ad_bf = nodes.tile([128, WPC], bf16)
        for w in range(WPC):
            for m in range(2):
                transpose_128(R_nm[:, w, ts(m, 128)], RT[:, m, ts(w, 128)])
                transpose_128(g_nm[:, w, ts(m, 128)], gT[:, m, ts(w, 128)])
            nc.sync.dma_start(out=ag2_in[ts(w, 128), 0:H], in_=R_nm[:, w, :])
            nc.sync.dma_start(out=ag2_in[ts(w, 128), H:2 * H], in_=g_nm[:, w, :])
            pa = psum.tile([128, 1], f32, tag="mm", bufs=4, name=f"pas{w}")
            for m in range(2):
                nc.tensor.matmul(pa[:], lhsT=gT[:, m, ts(w, 128)],
                                 rhs=sb_asrc[:, m:m + 1],
                                 start=(m == 0), stop=(m == 1))
            nc.vector.tensor_copy(as_nm[:, w:w + 1], pa[:])
            pd = psum.tile([128, 1], f32, tag="mm", bufs=4, name=f"pad{w}")
            for m in range(2):
                nc.tensor.matmul(pd[:], lhsT=gT[:, m, ts(w, 128)],
                                 rhs=sb_adst[:, m:m + 1],
                                 start=(m == 0), stop=(m == 1))
            nc.vector.tensor_copy(ad_bf[:, w:w + 1], pd[:])
            nc.sync.dma_start(out=ag2_in[ts(w, 128), 512:513],
                              in_=as_nm[:, w:w + 1])

        nc.gpsimd.collective_compute("AllGather", OP.bypass, replica_groups=RG,
                                     ins=[ag2_in.opt()], outs=[ag2_out.opt()])

        # ================= phase C + GAT edges =================
        u_nm = nodes.tile([128, WPC, H], f32, tag="nmA")
        glob_nm = nodes.tile([128, WPC, H], f32, tag="nmB")
        aggcp = [None] * WPC
        agggp = [None] * WPC
        for t in range(T_tot):
            w = t // T_w
            if t % T_w == 0:
                aggcp[w] = psum.tile([128, H], f32, tag="agg", bufs=2,
                                     name=f"aggC{w}")
                agggp[w] = psum.tile([128, H + 1], f32, tag="aggG", bufs=2,
                                     name=f"aggG{w}")
            gR = epool.tile([128, 520], f32, tag="gath2")
            nc.gpsimd.indirect_dma_start(
                out=gR[:], out_offset=None, in_=ag2_out[:, :],
                in_offset=IndirectOffsetOnAxis(ap=sb_src[:, t:t + 1], axis=0))
            sp = psum.tile([128, H], f32, tag="mm", bufs=4, name=f"sps{t}")
            nc.tensor.matmul(sp[:], lhsT=sb_ea7[:, ts(t, 128)], rhs=sb_w2c7[:],
                             start=True, stop=True)
            msg2 = epool.tile([128, H], bf16, tag="msg")
            nc.vector.tensor_tensor(msg2[:], gR[:, 0:H], sp[:], op=OP.mult)
            nc.tensor.matmul(aggcp[w][:], lhsT=sb_ohBC[:, ts(t, 128)],
                             rhs=msg2[:],
                             start=(t % T_w == 0), stop=(t % T_w == T_w - 1),
                             skip_group_check=True)
            adp = psum.tile([128, 1], f32, tag="mm", bufs=4, name=f"adp{t}")
            nc.tensor.matmul(adp[:], lhsT=sb_ohGT[:, ts(t, 128)],
                             rhs=ad_bf[:, w:w + 1], start=True, stop=True)
            eatt = epool.tile([128, 1], f32, tag="eatt")
            nc.vector.tensor_add(eatt[:], gR[:, 512:513], adp[:])
            el = epool.tile([128, 1], f32, tag="el")
            nc.vector.scalar_tensor_tensor(el[:], in0=eatt[:], scalar=SLOPE,
                                           in1=eatt[:], op0=OP.mult, op1=OP.max)
            ex = epool.tile([128, 1], f32, tag="ex")
            nc.scalar.activation(ex[:], el[:], AF.Exp)
            wmsg = epool.tile([128, H + 1], bf16, tag="wmsg")
            nc.vector.tensor_scalar(wmsg[:, 0:H], gR[:, H:2 * H], ex[:], None,
                                    op0=OP.mult)
            nc.vector.tensor_copy(wmsg[:, H:H + 1], ex[:])
            nc.tensor.matmul(agggp[w][:], lhsT=sb_ohG[:, ts(t, 128)],
                             rhs=wmsg[:],
                             start=(t % T_w == 0), stop=(t % T_w == T_w - 1),
                             skip_group_check=True)
            if t % T_w == T_w - 1:
                rec = epool.tile([128, 1], f32, tag="rec")
                nc.vector.reciprocal(rec[:], agggp[w][:, H:H + 1])
                nc.vector.tensor_scalar(glob_nm[:, w, :], agggp[w][:, 0:H],
                                        rec[:], None, op0=OP.mult)
                nc.vector.tensor_mul(u_nm[:, w, :], aggcp[w][:], h1_nm[:, w, :])

        # ================= tail: q =================
        uT = nodes.tile([128, 2, NPC], f32, tag="ftA")
        globT = nodes.tile([128, 2, NPC], f32, tag="ftB")
        for w in range(WPC):
            for m in range(2):
                transpose_128(uT[:, m, ts(w, 128)], u_nm[:, w, ts(m, 128)])
                transpose_128(globT[:, m, ts(w, 128)], glob_nm[:, w, ts(m, 128)])

        preT = nodes.tile([128, 2, NPC], f32, tag="h0T_re")
        for m in range(2):
            p = pmm()
            for kc in range(2):
                nc.tensor.matmul(p[:, :NPC], lhsT=sb_w3T[:, kc, ts(m, 128)],
                                 rhs=uT[:, kc, :], start=(kc == 0), stop=(kc == 1))
            lt = epool.tile([128, NPC], f32, tag="loc", bufs=2)
            nc.scalar.activation(lt[:], p[:, :NPC], AF.Identity,
                                 bias=sb_b3[:, m:m + 1])
            nc.vector.tensor_add(preT[:, m, :], lt[:], globT[:, m, :])

        t1T = nodes.tile([128, 2, NPC], f32, tag="aggT_re")
        for m in range(2):
            p = pmm()
            for kc in range(2):
                nc.tensor.matmul(p[:, :NPC], lhsT=sb_wl2T[:, kc, ts(m, 128)],
                                 rhs=preT[:, kc, :], start=(kc == 0), stop=(kc == 1))
            nc.scalar.copy(t1T[:, m, :], p[:, :NPC])

        qps = psum.tile([C, NPC], f32, tag="mm", bufs=4, name="qps_f")
        for kc in range(2):
            nc.tensor.matmul(qps[:], lhsT=sb_wl3T[:, kc, :], rhs=t1T[:, kc, :],
                             start=(kc == 0), stop=(kc == 1))
        qsb = nodes.tile([C, NPC], f32)
        nc.vector.tensor_scalar(qsb[:], qps[:], sb_qconst[:], None, op0=OP.add)

        q_nm = nodes.tile([128, WPC, C], bf16)
        for w in range(WPC):
            p = psum.tile([128, C], f32, tag="mm", bufs=4, name=f"qtr{w}")
            nc.tensor.transpose(p[:], qsb[:, ts(w, 128)], identity[:C, :C])
            nc.vector.tensor_copy(q_nm[:, w, :], p[:])
            nc.sync.dma_start(out=ag3_in[ts(w, 128), :], in_=q_nm[:, w, :])

        nc.gpsimd.collective_compute("AllGather", OP.bypass, replica_groups=RG,
                                     ins=[ag3_in.opt()], outs=[ag3_out.opt()])

        # ================= pairwise map =================
        patt = pwpool.tile([6, C * N], bf16)
        nc.sync.dma_start(out=patt[0:5, :], in_=d_pat5)
        patt3 = patt[5:6, :].rearrange("p (n c) -> p n c", c=C)
        nc.sync.dma_start(out=patt3, in_=ag3_out[:, :][None, :, :])

        lhsTq = pwpool.tile([6, NPC], bf16)
        nc.vector.memset(lhsTq[:], 1.0)
        nc.vector.tensor_copy(lhsTq[0:5, :], qsb[:])

        from concourse.bass import _add_dep_helper as add_dep
        big_by_itile = [[] for _ in range(WPC)]
        for it in range(WPC):
            for oc in range(NJC):
                ot = pwpool.tile([128, JCH], f32, tag="ot", bufs=3,
                                 name=f"ot{it}_{oc}")
                for s in range(C):
                    col = oc * JCH + s * 512
                    p = pmm()
                    nc.tensor.matmul(p[:], lhsT=lhsTq[:, ts(it, 128)],
                                     rhs=patt[:, col:col + 512],
                                     start=True, stop=True)
                    if s % 2 == 0:
                        nc.scalar.copy(ot[:, ts(s, 512)], p[:])
                    else:
                        nc.vector.tensor_copy(ot[:, ts(s, 512)], p[:])
                ot3 = ot[:].rearrange("p (j c) -> p j c", c=C)
                big = nc.sync.dma_start(
                    out=out3[ts(it, 128), oc * 512:(oc + 1) * 512, :], in_=ot3)
                big_by_itile[it].append(big)

        for it in range(WPC):
            ind = nc.gpsimd.indirect_dma_start(
                out=out_flat, out_offset=IndirectOffsetOnAxis(
                    ap=sb_diag[:, it:it + 1], axis=0),
                in_=neg1[:], in_offset=None)
            for b in big_by_itile[it]:
                add_dep(ind.ins, b.ins, reason="diag fixup after slab write")

    nc.compile()
    return nc


# ----------------------------------------------------------------------------
# entry point
# ----------------------------------------------------------------------------
def kernel(**inputs):
    from concourse import bass_utils

    g = {k: np.asarray(v) for k, v in inputs.items()}
    cores, T_w = _prep(g["edge_index"], g["edge_attr"])
    wts = _prep_weights(g)
    x = np.asarray(g["x"], np.float32)

    if T_w not in _cache:
        _cache[T_w] = _build(T_w)
    nc = _cache[T_w]

    in_maps = []
    for r in range(NCORES):
        m = dict(wts)
        m["xT"] = np.ascontiguousarray(x[r * NPC:(r + 1) * NPC].T)
        m.update(cores[r])
        in_maps.append(m)

    res = bass_utils.run_bass_kernel_spmd(nc, in_maps, core_ids=list(range(NCORES)))
    kernel._last_results = res
    out = np.concatenate([res.results[r]["out"] for r in range(NCORES)], axis=0)
    return out.reshape(N * N, C).astype(np.float32)


kernel._last_results = None


# revision 13
# speedup vs baseline: 1.1708x; 1.1708x over previous
"""Trainium2 Bass kernel for nn_GAT_WLN (GNN message passing, 8 NeuronCores).

Strategy (graph/data parallel per the sharding hint):
  - Nodes sharded 512/core; edges sharded by destination node.
  - The big [E, H+D] @ [H+D, H] edge matmul is factored algebraically:
    concat([h[src], ea]) @ W.T == (h @ Wa.T)[src] + (ea @ Wb.T), so edge work
    becomes node-level matmuls + indirect-DMA row gathers + one-hot
    scatter-matmuls (edges pre-sorted by dst into 128-node windows on host).
  - Self-loops for GAT are appended to the edge stream on host; softmax is
    computed without the max-subtraction (validated: |e| < ~2, safe in fp32).
  - P = h0 @ W1a.T is computed replicated (all 4096 nodes on every core) to
    avoid an AllGather; two AllGathers remain: [R|g|a_s] bf16 and q bf16.
  - All matmuls run in bf16 (fp32 PE matmuls cost 2 passes); PSUM stays f32.
  - Pairwise map q[x]+q[y]: per core a [512,4096,5] slab (42 MB) built by
    rank-6 matmuls against a host-precomputed interleave pattern, drained
    PSUM->SBUF on DVE+ACT, DMA'd out; diagonal -1 rows via indirect scatter.
"""
import numpy as np
import ml_dtypes

N, E = 4096, 32768
F, D, H, C = 82, 6, 256, 5
SLOPE = 0.2
NCORES = 8
NPC = N // NCORES          # 512 nodes per core
WIN = 128                  # dst window
WPC = NPC // WIN           # 4 windows per core

BF16 = ml_dtypes.bfloat16

_cache = {}


# ----------------------------------------------------------------------------
# host-side preprocessing
# ----------------------------------------------------------------------------
def _prep(edge_index, edge_attr):
    src = np.asarray(edge_index[0], dtype=np.int64)
    dst = np.asarray(edge_index[1], dtype=np.int64)
    ea = np.asarray(edge_attr, dtype=np.float32)

    order = np.argsort(dst, kind="stable")
    srcs, dsts = src[order], dst[order]
    eas = ea[order]

    counts = np.zeros((NCORES, WPC), dtype=np.int64)
    groups = [[None] * WPC for _ in range(NCORES)]
    gidx = dsts // WIN
    bounds = np.searchsorted(gidx, np.arange(NCORES * WPC + 1))
    for r in range(NCORES):
        for w in range(WPC):
            gw = r * WPC + w
            lo, hi = bounds[gw], bounds[gw + 1]
            groups[r][w] = (lo, hi)
            counts[r, w] = (hi - lo) + WIN   # + self loops

    T_w = int(-(-counts.max() // 128))
    EPW = T_w * 128
    EP = WPC * EPW
    T_tot = WPC * T_w

    cores = []
    for r in range(NCORES):
        src_sb = np.zeros((128, T_tot), np.int32)
        eaT7 = np.zeros((7, EP), np.float32)
        ohBC = np.zeros((128, T_tot * 128), np.float32)
        ohGAT = np.zeros((128, T_tot * 128), np.float32)
        ohGATT = np.zeros((128, T_tot * 128), np.float32)
        for w in range(WPC):
            lo, hi = groups[r][w]
            n_real = hi - lo
            base = w * EPW
            e_pos = base + np.arange(n_real)
            s_pos = base + n_real + np.arange(WIN)
            src_sb[e_pos % 128, e_pos // 128] = srcs[lo:hi]
            eaT7[:D, e_pos] = eas[lo:hi].T
            eaT7[6, e_pos] = 1.0
            nloc = (dsts[lo:hi] % WIN).astype(np.int64)
            ohBC[e_pos % 128, (e_pos // 128) * 128 + nloc] = 1.0
            ohGAT[e_pos % 128, (e_pos // 128) * 128 + nloc] = 1.0
            ohGATT[nloc, (e_pos // 128) * 128 + (e_pos % 128)] = 1.0
            self_ids = r * NPC + w * WIN + np.arange(WIN)
            src_sb[s_pos % 128, s_pos // 128] = self_ids
            nl = np.arange(WIN)
            ohGAT[s_pos % 128, (s_pos // 128) * 128 + nl] = 1.0
            ohGATT[nl, (s_pos // 128) * 128 + (s_pos % 128)] = 1.0
        iloc = np.arange(NPC)
        diag_sb = ((iloc * N) + (r * NPC + iloc)).astype(np.int32).reshape(WPC, 128).T
        cores.append(dict(
            src_sb=src_sb,
            eaT7=eaT7.astype(BF16),
            ohBC=ohBC.astype(BF16),
            ohGAT=ohGAT.astype(BF16),
            ohGATT=ohGATT.astype(BF16),
            diag_sb=np.ascontiguousarray(diag_sb),
        ))
    return cores, T_w


def _prep_weights(g):
    f32 = np.float32

    def c(a, dt=BF16):
        return np.ascontiguousarray(np.asarray(a, dtype=f32).astype(dt))

    def kchunks(wT, nk):
        K, M = wT.shape
        assert K == nk * 128
        return np.ascontiguousarray(
            np.asarray(wT, f32).reshape(nk, 128, M).transpose(1, 0, 2).astype(BF16))

    W1a = g["wl1_W1"][:, :H]
    W1b = g["wl1_W1"][:, H:]
    out = {}
    out["wlinT"] = c(g["W_lin"].T)                                   # [82,256]
    out["w1aT"] = kchunks(W1a.T, 2)
    out["w1bT7"] = c(np.vstack([W1b.T, g["wl1_b1"][None, :]]))
    out["w2T"] = kchunks(g["wl1_W2"].T, 4)
    out["b2c"] = np.ascontiguousarray(g["wl1_b2"].reshape(2, 128).T.astype(f32))
    out["w3T"] = kchunks(g["wl2_W3"].T, 2)
    out["b3c"] = np.ascontiguousarray(g["wl2_b3"].reshape(2, 128).T.astype(f32))
    out["w2c7"] = c(np.vstack([g["wl2_W2"].T, g["wl2_b2"][None, :]]))
    out["gatwT"] = kchunks(g["gat_W"].T, 2)
    out["asrcc"] = c(g["gat_asrc"].reshape(2, 128).T)
    out["adstc"] = c(g["gat_adst"].reshape(2, 128).T)
    out["wl2T"] = kchunks(g["W_lin2"].T, 2)
    out["wl3T"] = kchunks(g["W_lin3"].T, 2)
    out["qconstc"] = np.ascontiguousarray(
        (((g["gat_b"] @ g["W_lin2"].T) @ g["W_lin3"].T)[:, None]).astype(f32))
    out["pat5"] = np.ascontiguousarray(np.tile(np.eye(5, dtype=f32), N).astype(BF16))
    return out


# ----------------------------------------------------------------------------
# device program
# ----------------------------------------------------------------------------
def _build(T_w):
    import concourse.bass as bass
    import concourse.tile as tile
    from concourse import bacc, mybir
    from concourse.bass import IndirectOffsetOnAxis, ts
    from concourse.bass import _add_dep_helper as add_dep
    from concourse.masks import make_identity
    from contextlib import ExitStack

    f32 = mybir.dt.float32
    bf16 = mybir.dt.bfloat16
    i32 = mybir.dt.int32
    AF = mybir.ActivationFunctionType
    OP = mybir.AluOpType

    T_tot = WPC * T_w
    EP = T_tot * 128
    JCH = 512 * C          # 2560 output cols per chunk
    NJC = N // 512         # 8 chunks per row-tile
    NT_FULL = N // 128     # 32 node tiles (full graph)

    nc = bacc.Bacc("TRN2", target_bir_lowering=False, debug=False,
                   enable_asserts=False, num_devices=NCORES)

    def inp(name, shape, dt=bf16):
        return nc.dram_tensor(name, list(shape), dt, kind="ExternalInput").ap()

    d_xTf = inp("xTf", [F, N])
    d_xTl = inp("xTl", [F, NPC])
    d_wlinT = inp("wlinT", [F, H])
    d_w1aT = inp("w1aT", [128, 2, H])
    d_w1bT7 = inp("w1bT7", [7, H])
    d_w2T = inp("w2T", [128, 4, H])
    d_b2c = inp("b2c", [128, 2], f32)
    d_w3T = inp("w3T", [128, 2, H])
    d_b3c = inp("b3c", [128, 2], f32)
    d_w2c7 = inp("w2c7", [7, H])
    d_gatwT = inp("gatwT", [128, 2, H])
    d_asrcc = inp("asrcc", [128, 2])
    d_adstc = inp("adstc", [128, 2])
    d_wl2T = inp("wl2T", [128, 2, H])
    d_wl3T = inp("wl3T", [128, 2, C])
    d_qconstc = inp("qconstc", [C, 1], f32)
    d_pat5 = inp("pat5", [5, C * N])
    d_src = inp("src_sb", [128, T_tot], i32)
    d_ea7 = inp("eaT7", [7, EP])
    d_ohBC = inp("ohBC", [128, T_tot * 128])
    d_ohG = inp("ohGAT", [128, T_tot * 128])
    d_ohGT = inp("ohGATT", [128, T_tot * 128])
    d_diag = inp("diag_sb", [128, WPC], i32)

    out_h = nc.dram_tensor("out", [NPC * N, C], f32, kind="ExternalOutput")
    out_flat = out_h.ap()
    out3 = out_flat.rearrange("(i j) c -> i j c", i=NPC)

    with tile.TileContext(nc) as tc, ExitStack() as ctx:
        const = ctx.enter_context(tc.tile_pool(name="const", bufs=1))
        nodes = ctx.enter_context(tc.tile_pool(name="nodes", bufs=1))
        epool = ctx.enter_context(tc.tile_pool(name="edge", bufs=3))
        pwpool = ctx.enter_context(tc.tile_pool(name="pw", bufs=1))
        psum = ctx.enter_context(tc.tile_pool(name="psum", bufs=1, space="PSUM"))
        dram = ctx.enter_context(tc.tile_pool(name="dram", bufs=1, space="DRAM"))

        _n = [0]

        def pt(shape, tag="mm", dt=f32, bufs=4):
            _n[0] += 1
            return psum.tile(list(shape), dt, tag=tag, bufs=bufs,
                             name=f"ps{_n[0]}")

        def cload(name, ap, dt=bf16):
            t = const.tile(list(ap.shape), dt, name=name)
            nc.sync.dma_start(out=t[:], in_=ap)
            return t

        # early loads: what phase A needs
        sb_xTf = cload("sb_xTf", d_xTf)
        sb_xTl = cload("sb_xTl", d_xTl)
        sb_wlinT = cload("sb_wlinT", d_wlinT)
        sb_w1aT = cload("sb_w1aT", d_w1aT)
        identity = const.tile([128, 128], bf16)
        make_identity(nc, identity[:])

        def transpose_128(dst_ap, src_ap):
            p = pt([src_ap.shape[1], src_ap.shape[0]], dt=bf16)
            nc.tensor.transpose(p[:], src_ap,
                                identity[:src_ap.shape[0], :src_ap.shape[0]])
            nc.vector.tensor_copy(dst_ap, p[:])

        # ========== phase A: full h0, full P -> local DRAM (no collective) ==
        h0Tf = nodes.tile([128, 2, N], bf16)
        for m in range(2):
            for nck in range(NT_FULL // 4):            # chunks of 512 cols
                p = pt([128, 512])
                nc.tensor.matmul(p[:], lhsT=sb_wlinT[:, ts(m, 128)],
                                 rhs=sb_xTf[:, ts(nck, 512)], start=True, stop=True)
                nc.scalar.activation(h0Tf[:, m, ts(nck, 512)], p[:], AF.Relu)

        h0Tl = nodes.tile([128, 2, NPC], bf16)
        for m in range(2):
            p = pt([128, NPC])
            nc.tensor.matmul(p[:], lhsT=sb_wlinT[:, ts(m, 128)],
                             rhs=sb_xTl[:], start=True, stop=True)
            nc.scalar.activation(h0Tl[:, m, :], p[:], AF.Relu)

        P_dram = dram.tile([N, H], bf16)
        for nt in range(NT_FULL):
            p = pt([128, H])
            for kc in range(2):
                nc.tensor.matmul(p[:], lhsT=h0Tf[:, kc, ts(nt, 128)],
                                 rhs=sb_w1aT[:, kc, :],
                                 start=(kc == 0), stop=(kc == 1))
            pb = nodes.tile([128, H], bf16, tag="Pb", bufs=3, name=f"pb{nt}")
            if nt % 2 == 0:
                nc.vector.tensor_copy(pb[:], p[:])
            else:
                nc.scalar.copy(pb[:], p[:])
            nc.sync.dma_start(out=P_dram[ts(nt, 128), :], in_=pb[:])

        # remaining constant loads (overlap with phase A / edge loop ramp)
        sb_w1b7 = cload("sb_w1b7", d_w1bT7)
        sb_w2T = cload("sb_w2T", d_w2T)
        sb_b2 = cload("sb_b2", d_b2c, f32)
        sb_w3T = cload("sb_w3T", d_w3T)
        sb_b3 = cload("sb_b3", d_b3c, f32)
        sb_w2c7 = cload("sb_w2c7", d_w2c7)
        sb_gatwT = cload("sb_gatwT", d_gatwT)
        sb_asrc = cload("sb_asrc", d_asrcc)
        sb_adst = cload("sb_adst", d_adstc)
        sb_wl2T = cload("sb_wl2T", d_wl2T)
        sb_wl3T = cload("sb_wl3T", d_wl3T)
        sb_qconst = cload("sb_qconst", d_qconstc, f32)
        sb_src = cload("sb_src", d_src, i32)
        sb_ea7 = cload("sb_ea7", d_ea7)
        sb_ohBC = cload("sb_ohBC", d_ohBC)
        sb_ohG = cload("sb_ohG", d_ohG)
        sb_ohGT = cload("sb_ohGT", d_ohGT)
        sb_diag = cload("sb_diag", d_diag, i32)
        neg1 = const.tile([128, C], f32)
        nc.vector.memset(neg1[:], -1.0)

        ag2_in = dram.tile([NPC, 520], bf16)
        ag2_out = dram.tile([N, 520], bf16, addr_space="Shared")
        ag3_in = dram.tile([NPC, C], bf16)
        ag3_out = dram.tile([N, C], bf16, addr_space="Shared")
        RG = [list(range(NCORES))]

        # ========== phase B edges: msg -> agg -> h1 ==========
        agg_nm = nodes.tile([128, WPC, H], bf16)
        aggp = [None] * WPC
        for t in range(T_tot):
            w = t // T_w
            if t % T_w == 0:
                aggp[w] = pt([128, H], tag="agg", bufs=2)
            gathP = epool.tile([128, H], bf16, tag="gath", bufs=6)
            nc.gpsimd.indirect_dma_start(
                out=gathP[:], out_offset=None, in_=P_dram[:, :],
                in_offset=IndirectOffsetOnAxis(ap=sb_src[:, t:t + 1], axis=0))
            qp = pt([128, H])
            nc.tensor.matmul(qp[:], lhsT=sb_ea7[:, ts(t, 128)], rhs=sb_w1b7[:],
                             start=True, stop=True)
            tmp = epool.tile([128, H], f32, tag="tmpB")
            nc.vector.tensor_add(tmp[:], gathP[:], qp[:])
            msg = epool.tile([128, H], bf16, tag="msg")
            nc.scalar.activation(msg[:], tmp[:], AF.Relu)
            nc.tensor.matmul(aggp[w][:], lhsT=sb_ohBC[:, ts(t, 128)], rhs=msg[:],
                             start=(t % T_w == 0), stop=(t % T_w == T_w - 1),
                             skip_group_check=True)
            if t % T_w == T_w - 1:
                nc.scalar.copy(agg_nm[:, w, :], aggp[w][:])

        aggT = nodes.tile([128, 2, NPC], bf16)
        for w in range(WPC):
            for m in range(2):
                transpose_128(aggT[:, m, ts(w, 128)], agg_nm[:, w, ts(m, 128)])

        h1T = nodes.tile([128, 2, NPC], bf16)
        for m in range(2):
            p = pt([128, NPC])
            for kc in range(4):
                rhs = aggT[:, kc, :] if kc < 2 else h0Tl[:, kc - 2, :]
                nc.tensor.matmul(p[:], lhsT=sb_w2T[:, kc, ts(m, 128)],
                                 rhs=rhs, start=(kc == 0), stop=(kc == 3))
            nc.scalar.activation(h1T[:, m, :], p[:], AF.Relu,
                                 bias=sb_b2[:, m:m + 1])

        h1_nm = nodes.tile([128, WPC, H], bf16)
        for w in range(WPC):
            for m in range(2):
                transpose_128(h1_nm[:, w, ts(m, 128)], h1T[:, m, ts(w, 128)])

        # ========== R, g, a_s, a_d; AG2 ==========
        RT = nodes.tile([128, 2, NPC], bf16, tag="ftA")
        gT = nodes.tile([128, 2, NPC], bf16, tag="ftB")
        for m in range(2):
            p = pt([128, NPC])
            for kc in range(2):
                nc.tensor.matmul(p[:], lhsT=sb_w3T[:, kc, ts(m, 128)],
                                 rhs=h1T[:, kc, :], start=(kc == 0), stop=(kc == 1))
            nc.scalar.activation(RT[:, m, :], p[:], AF.Identity,
                                 bias=sb_b3[:, m:m + 1])
            p2 = pt([128, NPC])
            for kc in range(2):
                nc.tensor.matmul(p2[:], lhsT=sb_gatwT[:, kc, ts(m, 128)],
                                 rhs=h1T[:, kc, :], start=(kc == 0), stop=(kc == 1))
            nc.vector.tensor_copy(gT[:, m, :], p2[:])

        R_nm = nodes.tile([128, WPC, H], bf16, tag="nmA")
        g_nm = nodes.tile([128, WPC, H], bf16, tag="nmB")
        as_nm = nodes.tile([128, WPC], bf16)
        ad_bf = nodes.tile([128, WPC], bf16)
        for w in range(WPC):
            for m in range(2):
                transpose_128(R_nm[:, w, ts(m, 128)], RT[:, m, ts(w, 128)])
                transpose_128(g_nm[:, w, ts(m, 128)], gT[:, m, ts(w, 128)])
            nc.sync.dma_start(out=ag2_in[ts(w, 128), 0:H], in_=R_nm[:, w, :])
            nc.sync.dma_start(out=ag2_in[ts(w, 128), H:2 * H], in_=g_nm[:, w, :])
            pa = pt([128, 1])
            for m in range(2):
                nc.tensor.matmul(pa[:], lhsT=gT[:, m, ts(w, 128)],
                                 rhs=sb_asrc[:, m:m + 1],
                                 start=(m == 0), stop=(m == 1))
            nc.vector.tensor_copy(as_nm[:, w:w + 1], pa[:])
            pd = pt([128, 1])
            for m in range(2):
                nc.tensor.matmul(pd[:], lhsT=gT[:, m, ts(w, 128)],
                                 rhs=sb_adst[:, m:m + 1],
                                 start=(m == 0), stop=(m == 1))
            nc.vector.tensor_copy(ad_bf[:, w:w + 1], pd[:])
            nc.sync.dma_start(out=ag2_in[ts(w, 128), 512:513],
                              in_=as_nm[:, w:w + 1])

        nc.gpsimd.collective_compute("AllGather", OP.bypass, replica_groups=RG,
                                     ins=[ag2_in.opt()], outs=[ag2_out.opt()])

        # a_d per edge — no AG2 dependency, fills the collective stall
        ad_e_all = nodes.tile([128, T_tot], f32)
        for t in range(T_tot):
            w = t // T_w
            pd = pt([128, 1])
            nc.tensor.matmul(pd[:], lhsT=sb_ohGT[:, ts(t, 128)],
                             rhs=ad_bf[:, w:w + 1], start=True, stop=True)
            nc.vector.tensor_copy(ad_e_all[:, t:t + 1], pd[:])

        # ========== phase C + GAT edges ==========
        u_nm = nodes.tile([128, WPC, H], bf16, tag="nmA")
        glob_nm = nodes.tile([128, WPC, H], bf16, tag="nmB")
        aggcp = [None] * WPC
        agggp = [None] * WPC
        for t in range(T_tot):
            w = t // T_w
            if t % T_w == 0:
                aggcp[w] = pt([128, H], tag="agg", bufs=2)
                agggp[w] = pt([128, H + 1], tag="aggG", bufs=2)
            gR = epool.tile([128, 520], bf16, tag="gath2", bufs=6)
            nc.gpsimd.indirect_dma_start(
                out=gR[:], out_offset=None, in_=ag2_out[:, :],
                in_offset=IndirectOffsetOnAxis(ap=sb_src[:, t:t + 1], axis=0))
            sp = pt([128, H])
            nc.tensor.matmul(sp[:], lhsT=sb_ea7[:, ts(t, 128)], rhs=sb_w2c7[:],
                             start=True, stop=True)
            msg2 = epool.tile([128, H], bf16, tag="msg")
            nc.vector.tensor_tensor(msg2[:], gR[:, 0:H], sp[:], op=OP.mult)
            nc.tensor.matmul(aggcp[w][:], lhsT=sb_ohBC[:, ts(t, 128)],
                             rhs=msg2[:],
                             start=(t % T_w == 0), stop=(t % T_w == T_w - 1),
                             skip_group_check=True)
            eatt = epool.tile([128, 1], f32, tag="eatt")
            nc.vector.tensor_add(eatt[:], gR[:, 512:513], ad_e_all[:, t:t + 1])
            el = epool.tile([128, 1], f32, tag="el")
            nc.vector.scalar_tensor_tensor(el[:], in0=eatt[:], scalar=SLOPE,
                                           in1=eatt[:], op0=OP.mult, op1=OP.max)
            ex = epool.tile([128, 1], f32, tag="ex")
            nc.scalar.activation(ex[:], el[:], AF.Exp)
            wmsg = epool.tile([128, H + 1], bf16, tag="wmsg")
            nc.vector.tensor_scalar(wmsg[:, 0:H], gR[:, H:2 * H], ex[:], None,
                                    op0=OP.mult)
            nc.vector.tensor_copy(wmsg[:, H:H + 1], ex[:])
            nc.tensor.matmul(agggp[w][:], lhsT=sb_ohG[:, ts(t, 128)],
                             rhs=wmsg[:],
                             start=(t % T_w == 0), stop=(t % T_w == T_w - 1),
                             skip_group_check=True)
            if t % T_w == T_w - 1:
                rec = epool.tile([128, 1], f32, tag="rec")
                nc.vector.reciprocal(rec[:], agggp[w][:, H:H + 1])
                nc.vector.tensor_scalar(glob_nm[:, w, :], agggp[w][:, 0:H],
                                        rec[:], None, op0=OP.mult)
                nc.vector.tensor_mul(u_nm[:, w, :], aggcp[w][:], h1_nm[:, w, :])

        # ========== tail: q ==========
        uT = nodes.tile([128, 2, NPC], bf16, tag="ftA")
        globT = nodes.tile([128, 2, NPC], bf16, tag="ftB")
        for w in range(WPC):
            for m in range(2):
                transpose_128(uT[:, m, ts(w, 128)], u_nm[:, w, ts(m, 128)])
                transpose_128(globT[:, m, ts(w, 128)], glob_nm[:, w, ts(m, 128)])

        preT = nodes.tile([128, 2, NPC], bf16)
        for m in range(2):
            p = pt([128, NPC])
            for kc in range(2):
                nc.tensor.matmul(p[:], lhsT=sb_w3T[:, kc, ts(m, 128)],
                                 rhs=uT[:, kc, :], start=(kc == 0), stop=(kc == 1))
            lt = epool.tile([128, NPC], bf16, tag="loc", bufs=2)
            nc.scalar.activation(lt[:], p[:], AF.Identity,
                                 bias=sb_b3[:, m:m + 1])
            nc.vector.tensor_add(preT[:, m, :], lt[:], globT[:, m, :])

        t1T = nodes.tile([128, 2, NPC], bf16)
        for m in range(2):
            p = pt([128, NPC])
            for kc in range(2):
                nc.tensor.matmul(p[:], lhsT=sb_wl2T[:, kc, ts(m, 128)],
                                 rhs=preT[:, kc, :], start=(kc == 0), stop=(kc == 1))
            nc.scalar.copy(t1T[:, m, :], p[:])

        qps = pt([C, NPC])
        for kc in range(2):
            nc.tensor.matmul(qps[:], lhsT=sb_wl3T[:, kc, :], rhs=t1T[:, kc, :],
                             start=(kc == 0), stop=(kc == 1))
        qsb = nodes.tile([C, NPC], f32)
        nc.vector.tensor_scalar(qsb[:], qps[:], sb_qconst[:], None, op0=OP.add)
        qsb_bf = nodes.tile([C, NPC], bf16)
        nc.vector.tensor_copy(qsb_bf[:], qsb[:])

        q_nm = nodes.tile([128, WPC, C], bf16)
        for w in range(WPC):
            p = pt([128, C], dt=bf16)
            nc.tensor.transpose(p[:], qsb_bf[:, ts(w, 128)], identity[:C, :C])
            nc.vector.tensor_copy(q_nm[:, w, :], p[:])
            nc.sync.dma_start(out=ag3_in[ts(w, 128), :], in_=q_nm[:, w, :])

        nc.gpsimd.collective_compute("AllGather", OP.bypass, replica_groups=RG,
                                     ins=[ag3_in.opt()], outs=[ag3_out.opt()])

        # ========== pairwise map ==========
        patt = pwpool.tile([6, C * N], bf16)
        nc.sync.dma_start(out=patt[0:5, :], in_=d_pat5)
        patt3 = patt[5:6, :].rearrange("p (n c) -> p n c", c=C)
        nc.sync.dma_start(out=patt3, in_=ag3_out[:, :][None, :, :])

        lhsTq = pwpool.tile([6, NPC], bf16)
        nc.vector.memset(lhsTq[:], 1.0)
        nc.vector.tensor_copy(lhsTq[0:5, :], qsb[:])

        pw_tags = ["mm", "agg", "aggG", "mm", "agg"]
        pw_bufs = {"mm": 4, "agg": 2, "aggG": 2}
        big_by_itile = [[] for _ in range(WPC)]
        for it in range(WPC):
            for oc in range(NJC):
                ot = pwpool.tile([128, JCH], f32, tag="ot", bufs=3,
                                 name=f"ot{it}_{oc}")
                for s in range(C):
                    col = oc * JCH + s * 512
                    tag = pw_tags[s]
                    p = pt([128, 512], tag=tag, bufs=pw_bufs[tag])
                    nc.tensor.matmul(p[:], lhsT=lhsTq[:, ts(it, 128)],
                                     rhs=patt[:, col:col + 512],
                                     start=True, stop=True)
                    if s in (2, 4):
                        nc.scalar.copy(ot[:, ts(s, 512)], p[:])
                    else:
                        nc.vector.tensor_copy(ot[:, ts(s, 512)], p[:])
                ot3 = ot[:].rearrange("p (j c) -> p j c", c=C)
                big = nc.sync.dma_start(
                    out=out3[ts(it, 128), oc * 512:(oc + 1) * 512, :], in_=ot3)
                big_by_itile[it].append(big)

        for it in range(WPC):
            ind = nc.gpsimd.indirect_dma_start(
                out=out_flat, out_offset=IndirectOffsetOnAxis(
                    ap=sb_diag[:, it:it + 1], axis=0),
                in_=neg1[:], in_offset=None)
            for b in big_by_itile[it]:
                add_dep(ind.ins, b.ins, reason="diag fixup after slab write")

    nc.compile()
    return nc


# ----------------------------------------------------------------------------
# entry point
# ----------------------------------------------------------------------------
def kernel(**inputs):
    from concourse import bass_utils

    g = {k: np.asarray(v) for k, v in inputs.items()}
    cores, T_w = _prep(g["edge_index"], g["edge_attr"])
    wts = _prep_weights(g)
    x = np.asarray(g["x"], np.float32)
    xTf = np.ascontiguousarray(x.T.astype(BF16))

    if T_w not in _cache:
        _cache[T_w] = _build(T_w)
    nc = _cache[T_w]

    in_maps = []
    for r in range(NCORES):
        m = dict(wts)
        m["xTf"] = xTf
        m["xTl"] = np.ascontiguousarray(x[r * NPC:(r + 1) * NPC].T.astype(BF16))
        m.update(cores[r])
        in_maps.append(m)

    res = bass_utils.run_bass_kernel_spmd(nc, in_maps, core_ids=list(range(NCORES)))
    kernel._last_results = res
    out = np.concatenate([res.results[r]["out"] for r in range(NCORES)], axis=0)
    return out.reshape(N * N, C).astype(np.float32)


kernel._last_results = None
